# revision 25
# baseline (speedup 1.0000x reference)
"""AttnWindowPool Trainium2 kernel (v2: fused bf16 pipeline).

Math (per output row t, window w = t-3..t, per head h):
  k = e @ Wk, v = e @ Wv                       [L, H*128]
  s_h[t'] = q_h . k[t', h, :]                  (logits, shared across the 4 offsets)
  att = softmax over the window of s  ->  pooled = sum_w att_w * v[t-3+w]
Softmax-without-max identity (logits are O(5), exp can't overflow in fp32):
  pooled[t] = (sum_{s=t-3..t} E[s] * v[s]) / (sum_{s=t-3..t} E[s]),  E = exp(scale*s)
so the windowed attention is a constant banded matmul over U[s] = E[s]*v[s]
(plus the same banded sum of E for the denominator).
  out = pooled @ Wo + bo

Design: everything bf16 on the PE (1 col/cycle, no fp32r small-matmul
penalty, cheap LDWEIGHTS), e pre-transposed/packed on the host (kills
272 PE transposes + a DRAM round trip), logits fused into the
V-projection loop, and one fully fused per-128-row-block pipeline:
   v+s matmuls -> exp -> U=E*v -> banded pool + den -> 1/den scale
   -> one DMA-XBAR transpose (pooled -> pooledT, off the PE)
   -> out = pooledT.T @ Wo + bo (pipelined one block behind) -> DMA out.
Weights (Wv, Wo, folded Wq) stay SBUF-resident in bf16, streamed in
per-chunk behind prefetched etp blocks (DMA completion semaphores are
per-queue counters, so queue order is latency-critical).
All 16 transposes of a block ride ONE dma_start_transpose (3D out AP =
chunked transpose); splitting them across both hwdge queues races on
hardware (intermittent block corruption) - keep them on one queue.

Sharding: L split across 8 cores (2048 rows each) + one extra 128-row
"halo" block in front (previous core's last rows; zeros + E-mask for
core 0).
"""

import sys

sys.path.insert(0, "/opt/trn_rl_repo")

from contextlib import ExitStack

import numpy as np
import ml_dtypes

import concourse.bass as bass
import concourse.tile as tile
from concourse import mybir

# ---- problem constants (hardcoded per the grading contract) ----
L, D_IN, D_OUT, H, DH = 16384, 2048, 2048, 16, 128
W_LEFT = 3
N_CORES = 8
LC = L // N_CORES          # 2048 own rows per core
LH = LC + 128              # + one 128-row halo block in front
TB = LH // 128             # 17 t-blocks (block 0 = halo)
KC = D_IN // 128           # 16 contraction chunks
NC_ = D_OUT // 128         # 16 feature chunks
SCALE = 1.0 / float(np.sqrt(DH))
F32 = mybir.dt.float32
BF = mybir.dt.bfloat16
BF_NP = ml_dtypes.bfloat16
SPLIT_WAITS = True         # HW codegen needs 1-wait instrs; CoreSim can't parse NoOps


def _band_consts():
    d = np.zeros((128, 128), np.float32)   # diagT[s,t] = 1 if 0 <= t-s <= 3
    c = np.zeros((128, 128), np.float32)   # cornT[s,t] = 1 if s-t >= 125
    s = np.arange(128)[:, None]
    t = np.arange(128)[None, :]
    d[(t - s >= 0) & (t - s <= 3)] = 1.0
    c[(s - t) >= 125] = 1.0
    return d.astype(BF_NP), c.astype(BF_NP)


def build_nc():
    nc = bass.Bass("TRN2", target_bir_lowering=False, debug=False)

    etp_t = nc.dram_tensor("etp", [TB * 128, D_IN], BF, kind="ExternalInput")
    emask_t = nc.dram_tensor("emask", [128, 1], F32, kind="ExternalInput")
    wqp_t = nc.dram_tensor("wqp", [128, KC * H], BF, kind="ExternalInput")
    wvp_t = nc.dram_tensor("wvp", [128, KC * D_OUT], BF, kind="ExternalInput")
    wop_t = nc.dram_tensor("wop", [128, NC_ * D_OUT], BF, kind="ExternalInput")
    bo_t = nc.dram_tensor("bo", [D_OUT], F32, kind="ExternalInput")
    out_t = nc.dram_tensor("out", [LC, D_OUT], F32, kind="ExternalOutput")

    diag_np, corn_np = _band_consts()
    diag_d = nc.inline_tensor(diag_np, "band_diag")
    corn_d = nc.inline_tensor(corn_np, "band_corn")

    with tile.TileContext(nc) as tc:
        _kernel_body(tc, etp_t, emask_t, wqp_t, wvp_t, wop_t, bo_t, out_t,
                     diag_d, corn_d)
    if SPLIT_WAITS:
        _split_matmul_waits(nc)
    return nc


def _split_matmul_waits(nc):
    """walrus's LDWEIGHTS struct only has room for one sync-wait command, so
    a Matmult carrying >1 on_wait fails codegen ("Too many sync wait
    commands").  Move the extra waits onto NoOps just before the matmul on
    the PE queue (one wait per NoOp)."""
    for func in nc.m.functions:
        for blk in func.blocks:
            new_insts = []
            for inst in blk.instructions:
                si = getattr(inst, "sync_info", None)
                if (
                    si is not None
                    and si.on_wait
                    and len(si.on_wait) > 1
                    and not isinstance(inst, mybir.InstNoOp)
                ):
                    waits = list(si.on_wait)
                    for w in waits[:-1]:
                        nop = mybir.InstNoOp(
                            name=nc.get_next_instruction_name(),
                            ins=[],
                            outs=[],
                            sync_info=mybir.SyncInfo(on_wait=[w], on_update=[]),
                            bass_nofuse=True,
                            engine=inst.engine,
                        )
                        new_insts.append(nop)
                    inst.sync_info = mybir.SyncInfo(
                        on_wait=[waits[-1]], on_update=list(si.on_update)
                    )
                new_insts.append(inst)
            blk.instructions[:] = new_insts


def _kernel_body(tc, etp_t, emask_t, wqp_t, wvp_t, wop_t, bo_t, out_t,
                 diag_d, corn_d):
    nc = tc.nc
    Exp = mybir.ActivationFunctionType.Exp
    mult = mybir.AluOpType.mult
    add = mybir.AluOpType.add

    with ExitStack() as top:
        singles = top.enter_context(tc.tile_pool(name="singles", bufs=1))
        # ---- resident weights / consts ----
        diag_sb = singles.tile([128, 128], BF)
        nc.sync.dma_start(diag_sb[:], diag_d.ap())
        corn_sb = singles.tile([128, 128], BF)
        nc.sync.dma_start(corn_sb[:], corn_d.ap())
        emask_sb = singles.tile([128, 1], F32)
        nc.sync.dma_start(emask_sb[:], emask_t.ap())
        id_d = nc.inline_tensor(np.eye(128, dtype=BF_NP), "ident_bf")
        ident = singles.tile([128, 128], BF)
        nc.sync.dma_start(ident[:], id_d.ap())
        wq_sb = singles.tile([128, KC, H], BF)
        nc.sync.dma_start(wq_sb[:], wqp_t.ap().rearrange("p (a h) -> p a h", a=KC))
        bo_sb = singles.tile([128, D_OUT], F32)
        bo_bc = bass.AP(tensor=bo_t, offset=0, ap=[[0, 128], [1, D_OUT]])
        nc.gpsimd.dma_start(bo_sb[:], bo_bc)

        # ---- pools ----
        # PSUM budget (16KB = 8 banks; every tile rounds up to a bank):
        #   ps_v: v0,v1,v2 + sden  -> 4 banks
        #   ps_p: p0..p3           -> 4 banks
        # The 16 pooled->pooledT transposes stage through the p0/p1 banks
        # (fresh same-tag generations, bitcast to bf16) after the pooled
        # numerators were extracted from them.
        ps_v = top.enter_context(tc.tile_pool(name="ps_v", bufs=1, space="PSUM"))
        ps_p = top.enter_context(tc.tile_pool(name="ps_p", bufs=1, space="PSUM"))

        etp_pool = top.enter_context(tc.tile_pool(name="etp", bufs=3))

        def dma_etp(tb):
            t = etp_pool.tile([128, KC, 128], BF, tag="etp", name="etp")
            nc.sync.dma_start(
                t[:],
                etp_t[tb * 128 : (tb + 1) * 128, :].rearrange(
                    "p (a j) -> p a j", a=KC),
            )
            return t

        # etp blocks 0/1 are prefetched BEFORE the big weight streams: DMA
        # completion semaphores are per-queue counters, so anything queued
        # behind the 16MB of weights inherits their latency.
        etp_pre = {tb: dma_etp(tb) for tb in range(3)}
        # wv/wo streamed in per-kc chunks, alternating hwdge queues, so the
        # first v matmuls can start as soon as chunk 0 lands.  wv is split
        # twice as fine (half-kc chunks) to smooth the tb0 pacing stalls.
        wv_sb = singles.tile([128, KC, D_OUT], BF)
        for q in range(KC):
            for h in range(2):
                eng = nc.sync if (2 * q + h) % 2 == 0 else nc.scalar
                eng.dma_start(
                    wv_sb[:, q, h * 1024 : (h + 1) * 1024],
                    wvp_t[:, q * D_OUT + h * 1024 : q * D_OUT + (h + 1) * 1024],
                )
        wo_sb = singles.tile([128, NC_, D_OUT], BF)
        for q in range(NC_):
            eng = nc.sync if q % 2 == 0 else nc.scalar
            eng.dma_start(
                wo_sb[:, q, :],
                wop_t[:, q * D_OUT : (q + 1) * D_OUT],
            )

        u_pool = top.enter_context(tc.tile_pool(name="u", bufs=2))
        e32_pool = top.enter_context(tc.tile_pool(name="e32", bufs=2))
        ebf_pool = top.enter_context(tc.tile_pool(name="ebf", bufs=2))
        pn_pool = top.enter_context(tc.tile_pool(name="pn", bufs=2))
        pt_pool = top.enter_context(tc.tile_pool(name="pt", bufs=2))
        dv_pool = top.enter_context(tc.tile_pool(name="dv", bufs=2))
        ob_pool = top.enter_context(tc.tile_pool(name="ob", bufs=4))

        PORD = (3, 0, 1, 2)  # p3 first: its psum bank is recycled most

        def emit_D(pt, t_own):
            # out = pooledT.T @ Wo + bo  (pipelined one block behind)
            for p in PORD:
                op = ps_p.tile([128, 512], F32, tag=f"p{p}", name="op")
                for hc in range(NC_):
                    nc.tensor.matmul(
                        op[:], pt[:, hc, :],
                        wo_sb[:, hc, p * 512 : (p + 1) * 512],
                        start=(hc == 0), stop=(hc == NC_ - 1),
                    )
                osb = ob_pool.tile([128, 512], F32, tag="ob")
                nc.vector.tensor_tensor(
                    osb[:], op[:], bo_sb[:, p * 512 : (p + 1) * 512], add)
                nc.scalar.dma_start(
                    out_t[t_own * 128 : (t_own + 1) * 128,
                          p * 512 : (p + 1) * 512],
                    osb[:],
                )

        u_prev = None
        ebf_prev = None
        pt_prev = None
        for tb in range(TB):
            etp = etp_pre.pop(tb, None)
            if etp is None:
                etp = dma_etp(tb)
            # ---- V projection + logits: accumulate over kc per panel ----
            vtiles = {}
            vtiles[3] = ps_p.tile([128, 512], F32, tag="p0", name="vp3")
            for p in range(3):
                vtiles[p] = ps_v.tile([128, 512], F32, tag=f"v{p}", name=f"vp{p}")
            sden = ps_v.tile([128, 32], F32, tag="sden")
            if tb <= 2:
                # kc-outer: paces the matmuls with the streaming wv chunks
                # (they are still arriving through the first ~3 blocks)
                order = [(p, kc) for kc in range(KC) for p in PORD]
            else:
                # panel-outer: staggers the U extracts behind each panel
                order = [(p, kc) for p in PORD for kc in range(KC)]
            for p, kc in order:
                nc.tensor.matmul(
                    vtiles[p][:], etp[:, kc, :],
                    wv_sb[:, kc, p * 512 : (p + 1) * 512],
                    start=(kc == 0), stop=(kc == KC - 1),
                )
                if p == 3:  # logits ride the FIRST panel pass so the
                    nc.tensor.matmul(  # exp/U chain starts early
                        sden[:, 0:H], etp[:, kc, :], wq_sb[:, kc, :],
                        start=(kc == 0), stop=(kc == KC - 1),
                    )
            # ---- E = exp(scale*s) (masked on the halo block) ----
            e32 = e32_pool.tile([128, H], F32, tag="e32")
            nc.scalar.activation(e32[:], sden[:, 0:H], Exp, scale=SCALE)
            if tb == 0:
                nc.vector.tensor_tensor(
                    e32[:], e32[:],
                    emask_sb[:, 0, None].to_broadcast((128, H)), mult,
                )
            ebf = ebf_pool.tile([128, H], BF, tag="ebf")
            nc.scalar.copy(ebf[:], e32[:])
            # ---- U = E * v -> bf16 SBUF (extract panels; frees psum) ----
            u = u_pool.tile([128, D_OUT], BF, tag="u")
            for p in PORD:
                nc.vector.tensor_tensor(
                    u[:, p * 512 : (p + 1) * 512].rearrange(
                        "p (h d) -> p h d", d=DH),
                    vtiles[p][:].rearrange("p (h d) -> p h d", d=DH),
                    e32[:, p * 4 : (p + 1) * 4, None].to_broadcast((128, 4, DH)),
                    mult,
                )

            if tb >= 1:
                # ---- denominator first (feeds the reciprocal early) ----
                nc.tensor.matmul(sden[:, H : 2 * H], diag_sb[:], ebf[:],
                                 start=True, stop=False)
                nc.tensor.matmul(sden[:, H : 2 * H], corn_sb[:], ebf_prev[:],
                                 start=False, stop=True)
                dinv = dv_pool.tile([128, H], F32, tag="dinv")
                nc.vector.reciprocal(dinv[:], sden[:, H : 2 * H])
                # ---- banded pooling numerator panels ----
                prs = {}
                for p in PORD:
                    pr = ps_p.tile([128, 512], F32, tag=f"p{p}", name=f"pr{p}")
                    nc.tensor.matmul(pr[:], diag_sb[:],
                                     u[:, p * 512 : (p + 1) * 512],
                                     start=True, stop=False)
                    nc.tensor.matmul(pr[:], corn_sb[:],
                                     u_prev[:, p * 512 : (p + 1) * 512],
                                     start=False, stop=True)
                    prs[p] = pr
                # ---- pooled = num / den -> bf16 natural [t, n] ----
                pn = pn_pool.tile([128, D_OUT], BF, tag="pn")
                for p in PORD:
                    nc.vector.tensor_tensor(
                        pn[:, p * 512 : (p + 1) * 512].rearrange(
                            "p (h d) -> p h d", d=DH),
                        prs[p][:].rearrange("p (h d) -> p h d", d=DH),
                        dinv[:, p * 4 : (p + 1) * 4, None].to_broadcast(
                            (128, 4, DH)),
                        mult,
                    )
                # ---- pooled -> pooledT via ONE DMA XBAR transpose (off-PE).
                # The 3D out AP makes this a per-128-col-chunk transpose:
                # pt[d, hc, t] = pn[t, hc*128 + d].
                pt = pt_pool.tile([128, NC_, 128], BF, tag="pt")
                nc.sync.dma_start(pt[:], pn[:], transpose=True)
            if tb >= 2:
                emit_D(pt_prev, tb - 2)
            if tb >= 1:
                pt_prev = pt
            u_prev = u
            ebf_prev = ebf
        emit_D(pt_prev, TB - 2)


_NC_CACHE = None


def _get_nc():
    global _NC_CACHE
    if _NC_CACHE is None:
        _NC_CACHE = build_nc()
    return _NC_CACHE


def make_in_maps(e_seq, q_param, Wk, Wv, Wo, bo):
    e_seq = np.asarray(e_seq, np.float32)
    # fold Wk and q into the per-head logit weights (weight preprocessing):
    # s_h[t] = q_h . (e[t] @ Wk)[h*128:(h+1)*128] = e[t] . wqf[:, h]
    wqf = np.einsum(
        "khd,hd->kh",
        np.asarray(Wk, np.float32).reshape(D_IN, H, DH),
        np.asarray(q_param, np.float32),
    ).astype(np.float32)
    # packed (partition-major) weight layouts, bf16:
    #   w*[p, a, n] = W[a*128 + p, n]
    wqp = np.ascontiguousarray(
        wqf.reshape(KC, 128, H).transpose(1, 0, 2)).astype(BF_NP).reshape(128, KC * H)
    wvp = np.ascontiguousarray(
        np.asarray(Wv, np.float32).reshape(KC, 128, D_OUT).transpose(1, 0, 2)
    ).astype(BF_NP).reshape(128, KC * D_OUT)
    wop = np.ascontiguousarray(
        np.asarray(Wo, np.float32).reshape(NC_, 128, D_OUT).transpose(1, 0, 2)
    ).astype(BF_NP).reshape(128, NC_ * D_OUT)
    bo = np.ascontiguousarray(bo, np.float32)
    in_maps = []
    for c in range(N_CORES):
        s = c * LC
        if c == 0:
            eh = np.concatenate(
                [np.zeros((128, D_IN), np.float32), e_seq[:LC]], axis=0)
            em = np.zeros((128, 1), np.float32)
        else:
            eh = e_seq[s - 128 : s + LC]
            em = np.ones((128, 1), np.float32)
        # etp[(tb,p), (a,j)] = e[tb*128 + j, a*128 + p]
        etp = np.ascontiguousarray(
            eh.reshape(TB, 128, KC, 128).transpose(0, 3, 2, 1)
        ).astype(BF_NP).reshape(TB * 128, D_IN)
        in_maps.append({
            "etp": etp,
            "emask": em,
            "wqp": wqp,
            "wvp": wvp,
            "wop": wop,
            "bo": bo,
        })
    return in_maps


_RUNNER = None


def _get_runner():
    """Cached jitted 8-core runner (mirrors bass2jax.run_bass_via_pjrt)."""
    global _RUNNER
    if _RUNNER is not None:
        return _RUNNER
    import jax
    from jax.sharding import Mesh, PartitionSpec
    from jax.experimental.shard_map import shard_map
    from concourse import mybir as mb
    from concourse.bass2jax import (
        _bass_exec_p, install_neuronx_cc_hook, partition_id_tensor,
    )

    install_neuronx_cc_hook()
    nc = _get_nc()
    partition_name = (
        nc.partition_id_tensor.name if nc.partition_id_tensor else None
    )
    in_names, out_names, out_avals, zero_shapes = [], [], [], []
    for alloc in nc.m.functions[0].allocations:
        if not isinstance(alloc, mb.MemoryLocationSet):
            continue
        name = alloc.memorylocations[0].name
        if alloc.kind == "ExternalInput":
            if name != partition_name:
                in_names.append(name)
        elif alloc.kind == "ExternalOutput":
            out_names.append(name)
            shape = tuple(alloc.tensor_shape)
            dtype = mb.dt.np(alloc.dtype)
            out_avals.append(jax.core.ShapedArray(shape, dtype))
            zero_shapes.append((shape, dtype))
    n_params = len(in_names)
    n_outs = len(out_avals)
    all_names = in_names + out_names
    if partition_name is not None:
        all_names = all_names + [partition_name]
    donate = tuple(range(n_params, n_params + n_outs))

    def _body(*args):
        operands = list(args)
        if partition_name is not None:
            operands.append(partition_id_tensor())
        outs = _bass_exec_p.bind(
            *operands,
            out_avals=tuple(out_avals),
            in_names=tuple(all_names),
            out_names=tuple(out_names),
            lowering_input_output_aliases=(),
            sim_require_finite=True,
            sim_require_nnan=True,
            nc=nc,
        )
        return tuple(outs)

    devices = jax.devices()[:N_CORES]
    mesh = Mesh(np.asarray(devices), ("core",))
    sharded = jax.jit(
        shard_map(_body, mesh=mesh,
                  in_specs=(PartitionSpec("core"),) * (n_params + n_outs),
                  out_specs=(PartitionSpec("core"),) * n_outs,
                  check_rep=False),
        donate_argnums=donate, keep_unused=True,
    )

    from jax.sharding import NamedSharding

    shard = NamedSharding(mesh, PartitionSpec("core"))
    mk_zeros = jax.jit(
        lambda: tuple(
            jax.numpy.zeros((N_CORES * s[0], *s[1:]), d) for (s, d) in zero_shapes
        ),
        out_shardings=(shard,) * n_outs,
    )

    def place(in_maps):
        per_core = [[np.asarray(m[n]) for n in in_names] for m in in_maps]
        concat_in = [
            np.concatenate([per_core[c][i] for c in range(N_CORES)], axis=0)
            for i in range(n_params)
        ]
        return [jax.device_put(a, shard) for a in concat_in]

    def exec_placed(d_in, pull=True):
        out_arrs = sharded(*d_in, *mk_zeros())
        jax.block_until_ready(out_arrs)
        if not pull:
            return None
        out_np = [np.asarray(a) for a in out_arrs]
        return [
            {n: out_np[i].reshape(N_CORES, *zero_shapes[i][0])[c]
             for i, n in enumerate(out_names)}
            for c in range(N_CORES)
        ]

    def runner(in_maps):
        return exec_placed(place(in_maps))

    runner.place = place
    runner.exec_placed = exec_placed
    runner.sharded = sharded
    runner.mk_zeros = mk_zeros
    _RUNNER = runner
    return runner


def run(e_seq, q_param, Wk, Wv, Wo, bo, trace=False):
    in_maps = make_in_maps(e_seq, q_param, Wk, Wv, Wo, bo)
    runner = _get_runner()
    results = runner(in_maps)
    out = np.concatenate([results[c]["out"] for c in range(N_CORES)], axis=0)
    return out, results


def kernel(e_seq, q_param, Wk, Wv, Wo, bo):
    out, _ = run(e_seq, q_param, Wk, Wv, Wo, bo)
    return out.astype(np.float32)


# revision 26
# speedup vs baseline: 1.1867x; 1.1867x over previous
"""AttnWindowPool Trainium2 kernel (v2: fused bf16 pipeline).

Math (per output row t, window w = t-3..t, per head h):
  k = e @ Wk, v = e @ Wv                       [L, H*128]
  s_h[t'] = q_h . k[t', h, :]                  (logits, shared across the 4 offsets)
  att = softmax over the window of s  ->  pooled = sum_w att_w * v[t-3+w]
Softmax-without-max identity (logits are O(5), exp can't overflow in fp32):
  pooled[t] = (sum_{s=t-3..t} E[s] * v[s]) / (sum_{s=t-3..t} E[s]),  E = exp(scale*s)
so the windowed attention is a constant banded matmul over U[s] = E[s]*v[s]
(plus the same banded sum of E for the denominator).
  out = pooled @ Wo + bo

Design: everything bf16 on the PE (1 col/cycle, no fp32r small-matmul
penalty, cheap LDWEIGHTS), e pre-transposed/packed on the host (kills
272 PE transposes + a DRAM round trip), logits fused into the
V-projection loop, and one fully fused per-128-row-block pipeline:
   v+s matmuls -> exp -> U=E*v -> banded pool + den -> 1/den scale
   -> one DMA-XBAR transpose (pooled -> pooledT, off the PE)
   -> out = pooledT.T @ Wo + bo (pipelined one block behind) -> DMA out.
Weights (Wv, Wo, folded Wq) stay SBUF-resident in bf16, streamed in
per-chunk behind prefetched etp blocks (DMA completion semaphores are
per-queue counters, so queue order is latency-critical).
All 16 transposes of a block ride ONE dma_start_transpose (3D out AP =
chunked transpose); splitting them across both hwdge queues races on
hardware (intermittent block corruption) - keep them on one queue.

Sharding: L split across 8 cores (2048 rows each) + one extra 128-row
"halo" block in front (previous core's last rows; zeros + E-mask for
core 0).
"""

import sys

sys.path.insert(0, "/opt/trn_rl_repo")

from contextlib import ExitStack

import numpy as np
import ml_dtypes

import concourse.bass as bass
import concourse.tile as tile
from concourse import mybir

# ---- problem constants (hardcoded per the grading contract) ----
L, D_IN, D_OUT, H, DH = 16384, 2048, 2048, 16, 128
W_LEFT = 3
N_CORES = 8
LC = L // N_CORES          # 2048 own rows per core
LH = LC + 128              # + one 128-row halo block in front
TB = LH // 128             # 17 t-blocks (block 0 = halo)
KC = D_IN // 128           # 16 contraction chunks
NC_ = D_OUT // 128         # 16 feature chunks
SCALE = 1.0 / float(np.sqrt(DH))
F32 = mybir.dt.float32
BF = mybir.dt.bfloat16
BF_NP = ml_dtypes.bfloat16
SPLIT_WAITS = True         # HW codegen needs 1-wait instrs; CoreSim can't parse NoOps


def _band_consts():
    d = np.zeros((128, 128), np.float32)   # diagT[s,t] = 1 if 0 <= t-s <= 3
    c = np.zeros((128, 128), np.float32)   # cornT[s,t] = 1 if s-t >= 125
    s = np.arange(128)[:, None]
    t = np.arange(128)[None, :]
    d[(t - s >= 0) & (t - s <= 3)] = 1.0
    c[(s - t) >= 125] = 1.0
    return d.astype(BF_NP), c.astype(BF_NP)


def build_nc():
    nc = bass.Bass("TRN2", target_bir_lowering=False, debug=False)

    etp_t = nc.dram_tensor("etp", [TB * 128, D_IN], BF, kind="ExternalInput")
    emask_t = nc.dram_tensor("emask", [128, 1], F32, kind="ExternalInput")
    wqp_t = nc.dram_tensor("wqp", [128, KC * H], BF, kind="ExternalInput")
    wvp_t = nc.dram_tensor("wvp", [128, KC * D_OUT], BF, kind="ExternalInput")
    wop_t = nc.dram_tensor("wop", [128, NC_ * D_OUT], BF, kind="ExternalInput")
    bo_t = nc.dram_tensor("bo", [D_OUT], F32, kind="ExternalInput")
    out_t = nc.dram_tensor("out", [LC, D_OUT], F32, kind="ExternalOutput")

    diag_np, corn_np = _band_consts()
    diag_d = nc.inline_tensor(diag_np, "band_diag")
    corn_d = nc.inline_tensor(corn_np, "band_corn")

    with tile.TileContext(nc) as tc:
        _kernel_body(tc, etp_t, emask_t, wqp_t, wvp_t, wop_t, bo_t, out_t,
                     diag_d, corn_d)
    if SPLIT_WAITS:
        _split_matmul_waits(nc)
    return nc


def _split_matmul_waits(nc):
    """walrus's LDWEIGHTS struct only has room for one sync-wait command, so
    a Matmult carrying >1 on_wait fails codegen ("Too many sync wait
    commands").  Move the extra waits onto NoOps just before the matmul on
    the PE queue (one wait per NoOp)."""
    for func in nc.m.functions:
        for blk in func.blocks:
            new_insts = []
            for inst in blk.instructions:
                si = getattr(inst, "sync_info", None)
                if (
                    si is not None
                    and si.on_wait
                    and len(si.on_wait) > 1
                    and not isinstance(inst, mybir.InstNoOp)
                ):
                    waits = list(si.on_wait)
                    for w in waits[:-1]:
                        nop = mybir.InstNoOp(
                            name=nc.get_next_instruction_name(),
                            ins=[],
                            outs=[],
                            sync_info=mybir.SyncInfo(on_wait=[w], on_update=[]),
                            bass_nofuse=True,
                            engine=inst.engine,
                        )
                        new_insts.append(nop)
                    inst.sync_info = mybir.SyncInfo(
                        on_wait=[waits[-1]], on_update=list(si.on_update)
                    )
                new_insts.append(inst)
            blk.instructions[:] = new_insts


def _kernel_body(tc, etp_t, emask_t, wqp_t, wvp_t, wop_t, bo_t, out_t,
                 diag_d, corn_d):
    nc = tc.nc
    Exp = mybir.ActivationFunctionType.Exp
    mult = mybir.AluOpType.mult
    add = mybir.AluOpType.add

    with ExitStack() as top:
        singles = top.enter_context(tc.tile_pool(name="singles", bufs=1))
        # ---- resident weights / consts ----
        diag_sb = singles.tile([128, 128], BF)
        nc.sync.dma_start(diag_sb[:], diag_d.ap())
        corn_sb = singles.tile([128, 128], BF)
        nc.sync.dma_start(corn_sb[:], corn_d.ap())
        emask_sb = singles.tile([128, 1], F32)
        nc.sync.dma_start(emask_sb[:], emask_t.ap())
        id_d = nc.inline_tensor(np.eye(128, dtype=BF_NP), "ident_bf")
        ident = singles.tile([128, 128], BF)
        nc.sync.dma_start(ident[:], id_d.ap())
        wq_sb = singles.tile([128, KC, H], BF)
        nc.sync.dma_start(wq_sb[:], wqp_t.ap().rearrange("p (a h) -> p a h", a=KC))
        bo_sb = singles.tile([128, D_OUT], F32)
        bo_bc = bass.AP(tensor=bo_t, offset=0, ap=[[0, 128], [1, D_OUT]])
        nc.gpsimd.dma_start(bo_sb[:], bo_bc)

        # ---- pools ----
        # PSUM budget (16KB = 8 banks; every tile rounds up to a bank):
        #   ps_v: v0,v1,v2 + sden  -> 4 banks
        #   ps_p: p0..p3           -> 4 banks
        # The 16 pooled->pooledT transposes stage through the p0/p1 banks
        # (fresh same-tag generations, bitcast to bf16) after the pooled
        # numerators were extracted from them.
        ps_v = top.enter_context(tc.tile_pool(name="ps_v", bufs=1, space="PSUM"))
        ps_p = top.enter_context(tc.tile_pool(name="ps_p", bufs=1, space="PSUM"))

        etp_pool = top.enter_context(tc.tile_pool(name="etp", bufs=3))

        def dma_etp(tb):
            t = etp_pool.tile([128, KC, 128], BF, tag="etp", name="etp")
            nc.sync.dma_start(
                t[:],
                etp_t[tb * 128 : (tb + 1) * 128, :].rearrange(
                    "p (a j) -> p a j", a=KC),
            )
            return t

        # etp blocks 0/1 are prefetched BEFORE the big weight streams: DMA
        # completion semaphores are per-queue counters, so anything queued
        # behind the 16MB of weights inherits their latency.
        etp_pre = {tb: dma_etp(tb) for tb in range(3)}
        # wv/wo streamed in per-kc chunks, alternating hwdge queues, so the
        # first v matmuls can start as soon as chunk 0 lands.  wv is split
        # twice as fine (half-kc chunks) to smooth the tb0 pacing stalls.
        wv_sb = singles.tile([128, KC, D_OUT], BF)
        for q in range(KC):
            for h in range(2):
                eng = nc.sync if (2 * q + h) % 2 == 0 else nc.scalar
                eng.dma_start(
                    wv_sb[:, q, h * 1024 : (h + 1) * 1024],
                    wvp_t[:, q * D_OUT + h * 1024 : q * D_OUT + (h + 1) * 1024],
                )
        wo_sb = singles.tile([128, NC_, D_OUT], BF)
        for q in range(NC_):
            eng = nc.sync if q % 2 == 0 else nc.scalar
            eng.dma_start(
                wo_sb[:, q, :],
                wop_t[:, q * D_OUT : (q + 1) * D_OUT],
            )

        u_pool = top.enter_context(tc.tile_pool(name="u", bufs=2))
        e32_pool = top.enter_context(tc.tile_pool(name="e32", bufs=2))
        ebf_pool = top.enter_context(tc.tile_pool(name="ebf", bufs=2))
        pn_pool = top.enter_context(tc.tile_pool(name="pn", bufs=2))
        pt_pool = top.enter_context(tc.tile_pool(name="pt", bufs=2))
        dv_pool = top.enter_context(tc.tile_pool(name="dv", bufs=2))
        ob_pool = top.enter_context(tc.tile_pool(name="ob", bufs=4))

        PORD = (3, 0, 1, 2)  # p3 first: its psum bank is recycled most

        def emit_D(pt, t_own):
            # out = pooledT.T @ Wo + bo  (pipelined one block behind)
            for p in PORD:
                op = ps_p.tile([128, 512], F32, tag=f"p{p}", name="op")
                for hc in range(NC_):
                    nc.tensor.matmul(
                        op[:], pt[:, hc, :],
                        wo_sb[:, hc, p * 512 : (p + 1) * 512],
                        start=(hc == 0), stop=(hc == NC_ - 1),
                    )
                osb = ob_pool.tile([128, 512], F32, tag="ob")
                nc.vector.tensor_tensor(
                    osb[:], op[:], bo_sb[:, p * 512 : (p + 1) * 512], add)
                nc.scalar.dma_start(
                    out_t[t_own * 128 : (t_own + 1) * 128,
                          p * 512 : (p + 1) * 512],
                    osb[:],
                )

        u_prev = None
        ebf_prev = None
        pt_prev = None
        for tb in range(TB):
            etp = etp_pre.pop(tb, None)
            if etp is None:
                etp = dma_etp(tb)
            # ---- V projection + logits: accumulate over kc per panel ----
            vtiles = {}
            vtiles[3] = ps_p.tile([128, 512], F32, tag="p0", name="vp3")
            for p in range(3):
                vtiles[p] = ps_v.tile([128, 512], F32, tag=f"v{p}", name=f"vp{p}")
            sden = ps_v.tile([128, 32], F32, tag="sden")
            if tb == 0:
                # kc-outer: paces the matmuls with the streaming wv chunks.
                # (Tried tb<=2 too: 100us SLOWER — bunched U extracts stall
                # the band/D chain through the early pipeline. Keep tb==0.)
                order = [(p, kc) for kc in range(KC) for p in PORD]
            else:
                # panel-outer: staggers the U extracts behind each panel
                order = [(p, kc) for p in PORD for kc in range(KC)]
            for p, kc in order:
                nc.tensor.matmul(
                    vtiles[p][:], etp[:, kc, :],
                    wv_sb[:, kc, p * 512 : (p + 1) * 512],
                    start=(kc == 0), stop=(kc == KC - 1),
                )
                if p == 3:  # logits ride the FIRST panel pass so the
                    nc.tensor.matmul(  # exp/U chain starts early
                        sden[:, 0:H], etp[:, kc, :], wq_sb[:, kc, :],
                        start=(kc == 0), stop=(kc == KC - 1),
                    )
            # ---- E = exp(scale*s) (masked on the halo block) ----
            e32 = e32_pool.tile([128, H], F32, tag="e32")
            nc.scalar.activation(e32[:], sden[:, 0:H], Exp, scale=SCALE)
            if tb == 0:
                nc.vector.tensor_tensor(
                    e32[:], e32[:],
                    emask_sb[:, 0, None].to_broadcast((128, H)), mult,
                )
            ebf = ebf_pool.tile([128, H], BF, tag="ebf")
            nc.scalar.copy(ebf[:], e32[:])
            # ---- U = E * v -> bf16 SBUF (extract panels; frees psum) ----
            u = u_pool.tile([128, D_OUT], BF, tag="u")
            for p in PORD:
                nc.vector.tensor_tensor(
                    u[:, p * 512 : (p + 1) * 512].rearrange(
                        "p (h d) -> p h d", d=DH),
                    vtiles[p][:].rearrange("p (h d) -> p h d", d=DH),
                    e32[:, p * 4 : (p + 1) * 4, None].to_broadcast((128, 4, DH)),
                    mult,
                )

            if tb >= 1:
                # ---- denominator first (feeds the reciprocal early) ----
                nc.tensor.matmul(sden[:, H : 2 * H], diag_sb[:], ebf[:],
                                 start=True, stop=False)
                nc.tensor.matmul(sden[:, H : 2 * H], corn_sb[:], ebf_prev[:],
                                 start=False, stop=True)
                dinv = dv_pool.tile([128, H], F32, tag="dinv")
                nc.vector.reciprocal(dinv[:], sden[:, H : 2 * H])
                # ---- banded pooling numerator panels ----
                prs = {}
                for p in PORD:
                    pr = ps_p.tile([128, 512], F32, tag=f"p{p}", name=f"pr{p}")
                    nc.tensor.matmul(pr[:], diag_sb[:],
                                     u[:, p * 512 : (p + 1) * 512],
                                     start=True, stop=False)
                    nc.tensor.matmul(pr[:], corn_sb[:],
                                     u_prev[:, p * 512 : (p + 1) * 512],
                                     start=False, stop=True)
                    prs[p] = pr
                # ---- pooled = num / den -> bf16 natural [t, n] ----
                pn = pn_pool.tile([128, D_OUT], BF, tag="pn")
                for p in PORD:
                    nc.vector.tensor_tensor(
                        pn[:, p * 512 : (p + 1) * 512].rearrange(
                            "p (h d) -> p h d", d=DH),
                        prs[p][:].rearrange("p (h d) -> p h d", d=DH),
                        dinv[:, p * 4 : (p + 1) * 4, None].to_broadcast(
                            (128, 4, DH)),
                        mult,
                    )
                # ---- pooled -> pooledT via ONE DMA XBAR transpose (off-PE).
                # The 3D out AP makes this a per-128-col-chunk transpose:
                # pt[d, hc, t] = pn[t, hc*128 + d].
                pt = pt_pool.tile([128, NC_, 128], BF, tag="pt")
                nc.sync.dma_start(pt[:], pn[:], transpose=True)
            if tb >= 2:
                emit_D(pt_prev, tb - 2)
            if tb >= 1:
                pt_prev = pt
            u_prev = u
            ebf_prev = ebf
        emit_D(pt_prev, TB - 2)


_NC_CACHE = None


def _get_nc():
    global _NC_CACHE
    if _NC_CACHE is None:
        _NC_CACHE = build_nc()
    return _NC_CACHE


def make_in_maps(e_seq, q_param, Wk, Wv, Wo, bo):
    e_seq = np.asarray(e_seq, np.float32)
    # fold Wk and q into the per-head logit weights (weight preprocessing):
    # s_h[t] = q_h . (e[t] @ Wk)[h*128:(h+1)*128] = e[t] . wqf[:, h]
    wqf = np.einsum(
        "khd,hd->kh",
        np.asarray(Wk, np.float32).reshape(D_IN, H, DH),
        np.asarray(q_param, np.float32),
    ).astype(np.float32)
    # packed (partition-major) weight layouts, bf16:
    #   w*[p, a, n] = W[a*128 + p, n]
    wqp = np.ascontiguousarray(
        wqf.reshape(KC, 128, H).transpose(1, 0, 2)).astype(BF_NP).reshape(128, KC * H)
    wvp = np.ascontiguousarray(
        np.asarray(Wv, np.float32).reshape(KC, 128, D_OUT).transpose(1, 0, 2)
    ).astype(BF_NP).reshape(128, KC * D_OUT)
    wop = np.ascontiguousarray(
        np.asarray(Wo, np.float32).reshape(NC_, 128, D_OUT).transpose(1, 0, 2)
    ).astype(BF_NP).reshape(128, NC_ * D_OUT)
    bo = np.ascontiguousarray(bo, np.float32)
    in_maps = []
    for c in range(N_CORES):
        s = c * LC
        if c == 0:
            eh = np.concatenate(
                [np.zeros((128, D_IN), np.float32), e_seq[:LC]], axis=0)
            em = np.zeros((128, 1), np.float32)
        else:
            eh = e_seq[s - 128 : s + LC]
            em = np.ones((128, 1), np.float32)
        # etp[(tb,p), (a,j)] = e[tb*128 + j, a*128 + p]
        etp = np.ascontiguousarray(
            eh.reshape(TB, 128, KC, 128).transpose(0, 3, 2, 1)
        ).astype(BF_NP).reshape(TB * 128, D_IN)
        in_maps.append({
            "etp": etp,
            "emask": em,
            "wqp": wqp,
            "wvp": wvp,
            "wop": wop,
            "bo": bo,
        })
    return in_maps


_RUNNER = None


def _get_runner():
    """Cached jitted 8-core runner (mirrors bass2jax.run_bass_via_pjrt)."""
    global _RUNNER
    if _RUNNER is not None:
        return _RUNNER
    import jax
    from jax.sharding import Mesh, PartitionSpec
    from jax.experimental.shard_map import shard_map
    from concourse import mybir as mb
    from concourse.bass2jax import (
        _bass_exec_p, install_neuronx_cc_hook, partition_id_tensor,
    )

    install_neuronx_cc_hook()
    nc = _get_nc()
    partition_name = (
        nc.partition_id_tensor.name if nc.partition_id_tensor else None
    )
    in_names, out_names, out_avals, zero_shapes = [], [], [], []
    for alloc in nc.m.functions[0].allocations:
        if not isinstance(alloc, mb.MemoryLocationSet):
            continue
        name = alloc.memorylocations[0].name
        if alloc.kind == "ExternalInput":
            if name != partition_name:
                in_names.append(name)
        elif alloc.kind == "ExternalOutput":
            out_names.append(name)
            shape = tuple(alloc.tensor_shape)
            dtype = mb.dt.np(alloc.dtype)
            out_avals.append(jax.core.ShapedArray(shape, dtype))
            zero_shapes.append((shape, dtype))
    n_params = len(in_names)
    n_outs = len(out_avals)
    all_names = in_names + out_names
    if partition_name is not None:
        all_names = all_names + [partition_name]
    donate = tuple(range(n_params, n_params + n_outs))

    def _body(*args):
        operands = list(args)
        if partition_name is not None:
            operands.append(partition_id_tensor())
        outs = _bass_exec_p.bind(
            *operands,
            out_avals=tuple(out_avals),
            in_names=tuple(all_names),
            out_names=tuple(out_names),
            lowering_input_output_aliases=(),
            sim_require_finite=True,
            sim_require_nnan=True,
            nc=nc,
        )
        return tuple(outs)

    devices = jax.devices()[:N_CORES]
    mesh = Mesh(np.asarray(devices), ("core",))
    sharded = jax.jit(
        shard_map(_body, mesh=mesh,
                  in_specs=(PartitionSpec("core"),) * (n_params + n_outs),
                  out_specs=(PartitionSpec("core"),) * n_outs,
                  check_rep=False),
        donate_argnums=donate, keep_unused=True,
    )

    from jax.sharding import NamedSharding

    shard = NamedSharding(mesh, PartitionSpec("core"))
    mk_zeros = jax.jit(
        lambda: tuple(
            jax.numpy.zeros((N_CORES * s[0], *s[1:]), d) for (s, d) in zero_shapes
        ),
        out_shardings=(shard,) * n_outs,
    )

    def place(in_maps):
        per_core = [[np.asarray(m[n]) for n in in_names] for m in in_maps]
        concat_in = [
            np.concatenate([per_core[c][i] for c in range(N_CORES)], axis=0)
            for i in range(n_params)
        ]
        return [jax.device_put(a, shard) for a in concat_in]

    def exec_placed(d_in, pull=True):
        out_arrs = sharded(*d_in, *mk_zeros())
        jax.block_until_ready(out_arrs)
        if not pull:
            return None
        out_np = [np.asarray(a) for a in out_arrs]
        return [
            {n: out_np[i].reshape(N_CORES, *zero_shapes[i][0])[c]
             for i, n in enumerate(out_names)}
            for c in range(N_CORES)
        ]

    def runner(in_maps):
        return exec_placed(place(in_maps))

    runner.place = place
    runner.exec_placed = exec_placed
    runner.sharded = sharded
    runner.mk_zeros = mk_zeros
    _RUNNER = runner
    return runner


def run(e_seq, q_param, Wk, Wv, Wo, bo, trace=False):
    in_maps = make_in_maps(e_seq, q_param, Wk, Wv, Wo, bo)
    runner = _get_runner()
    results = runner(in_maps)
    out = np.concatenate([results[c]["out"] for c in range(N_CORES)], axis=0)
    return out, results


def kernel(e_seq, q_param, Wk, Wv, Wo, bo):
    out, _ = run(e_seq, q_param, Wk, Wv, Wo, bo)
    return out.astype(np.float32)
